# revision 23
# baseline (speedup 1.0000x reference)
"""Causal multi-head attention on 8 Trainium2 NeuronCores.

Problem (hardcoded): x [4, 2048, 1024] fp32, W_qkv [1024, 3072], b_qkv [3072],
W_o [1024, 1024], b_o [1024]; 16 heads, head_dim 64.

Sharding: 8 cores = 4 batches x 2 head-groups (8 heads each). Each core
computes QKV projection for its (batch, head-group), causal attention for its
8 heads, and a partial out-projection [2048, 1024]. Host sums the two
head-group partials per batch and adds b_o.

Kernel strategy (per core, everything in the "transposed" domain):
  - x strip [512, 1024] -> PE-transpose -> xT [128, 8ds, 512]
  - QT/KTz = W^T x^T via matmul(lhsT=W_tile, rhs=xT) -> [n-feature, s] layout.
    KTz is zero-padded per head to a full 128-partition contraction: head h
    keeps its 64 rows, the sibling head's rows are zeros, so the A^T matmul
    runs K=128 (keeps the PE activity monitor at full clock).
  - V natural = matmul(lhsT=xT_tile, rhs=Wv) -> [s, n] layout, stored per
    128-row tile as [128, head, 65] with a ones column at 64 (bf16)
  - A^T[sk, sq] = matmul(lhsT=KTz_h tile, rhs=QT);  exp via ScalarE
    (scale=1/8, no max-subtraction: |aff| < 3 for this data);
    causal mask on diagonal blocks via gpsimd.affine_select
  - O^T + denominator accumulate: matmul(lhsT=Vn[128,65], rhs=expA) ->
    psum[65,512]; normalize via reciprocal_approx_fast + PE row-broadcast
  - out partial = matmul(lhsT=OT tile, rhs=W_o tiles) -> [s, e], DMA out
Projection/out-proj matmuls run as float32r (single-pass "HIGH" fp32 mode);
attention matmuls run bf16.
"""

import ml_dtypes
import numpy as np

import concourse.bass as bass
from concourse import bacc
import concourse.mybir as mybir
from concourse.bass_utils import run_bass_kernel_spmd
from concourse.masks import make_identity
from concourse.tile import TileContext

B, S, D = 4, 2048, 1024
H, HD = 16, 64
G = 2                  # head groups (cores per batch)
HPG = H // G           # 8 heads per core
NG = HPG * HD          # 512 qkv feature columns per core
N_CORES = 8
STRIP = 512            # sq strip width (and matmul moving dim)
NSTRIP = S // STRIP    # 4
DS = D // 128          # 8 contraction subtiles for the projections
FP32 = mybir.dt.float32
R32 = mybir.dt.float32r
BF16 = mybir.dt.bfloat16
AF = mybir.ActivationFunctionType


def build_bass(dbg=False):
    nc = bacc.Bacc("TRN2")

    x_d = nc.dram_tensor("x", [S, D], FP32, kind="ExternalInput")
    wq_d = nc.dram_tensor("wq", [D, NG], R32, kind="ExternalInput")
    wk_d = nc.dram_tensor("wk", [D, NG], R32, kind="ExternalInput")
    wv_d = nc.dram_tensor("wv", [D, NG], R32, kind="ExternalInput")
    bqk_d = nc.dram_tensor("bqk", [128, 8], FP32, kind="ExternalInput")
    onesr_d = nc.dram_tensor("onesr", [1, 128], R32, kind="ExternalInput")
    mask_d = nc.dram_tensor("mask", [128, 4, STRIP], BF16, kind="ExternalInput")
    bv_d = nc.dram_tensor("bv", [1, NG], R32, kind="ExternalInput")
    wo_d = nc.dram_tensor("wo", [NG, D], R32, kind="ExternalInput")
    out_d = nc.dram_tensor("out", [S, D], FP32, kind="ExternalOutput")

    with TileContext(nc) as tc:
        with (
            tc.tile_pool(name="const", bufs=1) as const,
            tc.tile_pool(name="persist", bufs=1) as persist,
            tc.tile_pool(name="work", bufs=2) as work,
            tc.tile_pool(name="psum", bufs=2, space="PSUM") as psum,
        ):
            ident = const.tile([128, 128], FP32, name="ident")
            make_identity(nc, ident)
            ones1x128 = const.tile([1, 128], R32, name="ones1x128")
            nc.sync.dma_start(ones1x128, onesr_d[:, :])
            ones1x64 = ones1x128[:, 0:64]

            mask_sb = const.tile([128, 4, STRIP], BF16, name="mask_sb")
            bqk_sb = const.tile([128, 8], FP32, name="bqk_sb")
            nc.sync.dma_start(bqk_sb, bqk_d[:, :])
            bv_sb = const.tile([1, NG], R32, name="bv_sb")
            nc.sync.dma_start(bv_sb, bv_d[:, :])
            wo_sb = const.tile([128, 4, D], R32, name="wo_sb")
            nc.sync.dma_start(wo_sb, wo_d[:, :].rearrange("(ns p) e -> p ns e", p=128))
            wv_sb = const.tile([128, DS, NG], R32, name="wv_sb")
            nc.sync.dma_start(wv_sb, wv_d[:, :].rearrange("(ds p) n -> p ds n", p=128))

            # Persistent zero-padded K^T per head and V tiles (both bf16)
            KTz = persist.tile([128, HPG, S], BF16, name="KTz")
            for h in range(HPG):
                zrow = 64 if h % 2 == 0 else 0
                nc.vector.memset(KTz[zrow:zrow + 64, h, :], 0.0)
            Vn = persist.tile([128, S // 128, HPG, HD + 1], BF16, name="Vn")
            nc.vector.memset(Vn[:, :, :, HD], 1.0)

            for i in range(NSTRIP):
                s0 = i * STRIP

                # ---- transpose x strip -> xT [128(d), ds, 512(s)] ----
                xT = work.tile([128, DS, STRIP], R32, name="xT", tag="xT",
                               bufs=1)
                for st in range(4):
                    xrow = work.tile([128, D], FP32, name="xrow",
                                     tag="xrow", bufs=3)
                    nc.sync.dma_start(
                        xrow, x_d[s0 + st * 128:s0 + (st + 1) * 128, :])
                    for ds in range(DS):
                        pst = psum.tile([128, 128], FP32, name="pst", tag="psA",
                                        bufs=2)
                        nc.tensor.transpose(
                            pst, xrow[:, ds * 128:(ds + 1) * 128], ident)
                        nc.vector.tensor_copy(xT[:, ds, st * 128:(st + 1) * 128], pst)

                if i == 0:
                    nc.sync.dma_start(bqk_sb, bqk_d[:, :])
                    nc.sync.dma_start(mask_sb, mask_d[:, :, :])
                    nc.sync.dma_start(ones1x128, onesr_d[:, :])
                    nc.sync.dma_start(bv_sb, bv_d[:, :])

                # ---- Q^T (strip) and zero-padded K^T (persistent) ----
                QT = work.tile([128, 4, STRIP], BF16, name="QT", tag="QT")
                for which, (w_d, bcol0) in enumerate(((wq_d, 0), (wk_d, 4))):
                    wt = work.tile([128, DS, NG], R32, name="wt",
                                   tag="wt", bufs=2)
                    w_r = w_d[:, :].rearrange("(ds p) n -> p ds n", p=128)
                    nc.sync.dma_start(wt[:, :, 0:NG // 2], w_r[:, :, 0:NG // 2])
                    nc.sync.dma_start(wt[:, :, NG // 2:], w_r[:, :, NG // 2:])
                    for nb in range(4):
                        ps = psum.tile([128, STRIP], FP32, name="ps", tag="ps_mm",
                                       bufs=2)
                        for ds in range(DS):
                            nc.tensor.matmul(
                                ps, lhsT=wt[:, ds, nb * 128:(nb + 1) * 128],
                                rhs=xT[:, ds],
                                start=(ds == 0), stop=(ds == DS - 1))
                        bcol = bqk_sb[:, bcol0 + nb:bcol0 + nb + 1]
                        if which == 0:
                            nc.vector.tensor_scalar_add(QT[:, nb, :], ps, bcol)
                        else:
                            nc.vector.tensor_scalar_add(
                                KTz[0:64, 2 * nb, s0:s0 + STRIP],
                                ps[0:64, :], bcol[0:64, :])
                            nc.vector.tensor_scalar_add(
                                KTz[64:128, 2 * nb + 1, s0:s0 + STRIP],
                                ps[64:128, :], bcol[64:128, :])

                # ---- V natural-layout projection (bias via rank-1 mm) ----
                if i == 0:
                    nc.sync.dma_start(
                        wv_sb, wv_d[:, :].rearrange("(ds p) n -> p ds n", p=128))
                for st in range(4):
                    stg = i * 4 + st
                    psv = psum.tile([128, STRIP], FP32, name="psv", tag="ps_mm",
                                    bufs=2)
                    nc.tensor.matmul(psv, lhsT=ones1x128, rhs=bv_sb,
                                     start=True, stop=False)
                    for ds in range(DS):
                        nc.tensor.matmul(
                            psv,
                            lhsT=xT[:, ds, st * 128:(st + 1) * 128],
                            rhs=wv_sb[:, ds],
                            start=False, stop=(ds == DS - 1))
                    nc.vector.tensor_copy(
                        Vn[:, stg, :, 0:HD],
                        psv.rearrange("p (h d) -> p h d", d=HD))

                # ---- causal attention for this sq strip ----
                if i == 0:
                    nc.sync.dma_start(
                        wo_sb, wo_d[:, :].rearrange("(ns p) e -> p ns e", p=128))
                nsk = 4 * (i + 1)
                OT = work.tile([128, 4, STRIP], R32, name="OT", tag="OT")
                for h in range(HPG):
                    prow = (h % 2) * 64
                    nsub = h // 2
                    psO = psum.tile([128, STRIP], FP32, name="psO", tag="psO")
                    for sk0 in range(0, nsk, 2):
                        psA = psum.tile([128, 2, STRIP], FP32, name="psA",
                                        tag="psA", bufs=2)
                        for o in range(2):
                            sk = sk0 + o
                            nc.tensor.matmul(
                                psA[:, o, :],
                                lhsT=KTz[:, h, sk * 128:(sk + 1) * 128],
                                rhs=QT[:, nsub, :],
                                start=True, stop=True, skip_group_check=True)
                        expA = work.tile([128, 2, STRIP], BF16, name="expA",
                                         tag="expA", bufs=4)
                        nc.scalar.activation(expA, psA, AF.Exp, scale=0.125)
                        j0 = sk0 - 4 * i
                        if j0 >= 0:
                            # zero where sq_in_strip < 128*(j0+o) + p  (causal)
                            nc.vector.tensor_mul(expA, expA,
                                                 mask_sb[:, j0:j0 + 2, :])
                        for o in range(2):
                            sk = sk0 + o
                            nc.tensor.matmul(psO[0:HD + 1, :],
                                             lhsT=Vn[:, sk, h, :],
                                             rhs=expA[:, o, :],
                                             start=(sk == 0), stop=(sk == nsk - 1))
                    den1 = work.tile([1, STRIP], FP32, name="den1",
                                     tag="den1", bufs=2)
                    nc.vector.tensor_copy(den1, psO[HD:HD + 1, :])
                    recip = work.tile([1, STRIP], FP32, name="recip",
                                      tag="recip", bufs=2)
                    nc.vector.reciprocal_approx_fast(recip, den1)
                    recip_r = work.tile([1, STRIP], R32, name="recip_r",
                                        tag="recip_r", bufs=2)
                    nc.vector.tensor_copy(recip_r, recip)
                    psB = psum.tile([64, STRIP], FP32, name="psB", tag="ps_mm",
                                    bufs=2)
                    nc.tensor.matmul(psB, lhsT=ones1x64, rhs=recip_r,
                                     start=True, stop=True)
                    bcast = work.tile([64, STRIP], FP32, name="bcast",
                                      tag="bcast", bufs=2)
                    nc.vector.tensor_copy(bcast, psB)
                    nc.vector.tensor_mul(OT[prow:prow + 64, nsub, :],
                                         psO[0:HD, :], bcast)

                # ---- partial out-projection for this strip ----
                for st in range(4):
                    ob = work.tile([128, D], FP32, name="ob", tag="ob",
                                   bufs=2)
                    for ec in range(2):
                        pso = psum.tile([128, STRIP], FP32, name="pso", tag="ps_mm",
                                        bufs=2)
                        for ns in range(4):
                            nc.tensor.matmul(
                                pso,
                                lhsT=OT[:, ns, st * 128:(st + 1) * 128],
                                rhs=wo_sb[:, ns, ec * 512:(ec + 1) * 512],
                                start=(ns == 0), stop=(ns == 3))
                        nc.vector.tensor_copy(ob[:, ec * 512:(ec + 1) * 512], pso)
                    nc.sync.dma_start(
                        out_d[s0 + st * 128:s0 + (st + 1) * 128, :], ob)
    nc.compile()
    return nc


_CACHE = {}


def _causal_masks():
    # mask[p, j, f] = 1.0 if f >= 128*j + p else 0  (keep sk <= sq)
    p = np.arange(128)[:, None, None]
    j = np.arange(4)[None, :, None]
    f = np.arange(STRIP)[None, None, :]
    return (f >= 128 * j + p).astype(np.float32).astype(ml_dtypes.bfloat16)


def kernel(x, W_qkv, b_qkv, W_o, b_o):
    x = np.ascontiguousarray(np.asarray(x, dtype=np.float32))
    W_qkv = np.asarray(W_qkv, dtype=np.float32)
    b_qkv = np.asarray(b_qkv, dtype=np.float32)
    W_o = np.asarray(W_o, dtype=np.float32)
    b_o = np.asarray(b_o, dtype=np.float32)

    if "nc" not in _CACHE:
        _CACHE["nc"] = build_bass()
    nc = _CACHE["nc"]

    in_maps = []
    for c in range(N_CORES):
        b, g = c // G, c % G
        n0 = g * NG
        bq = b_qkv[n0:n0 + NG]
        bk = b_qkv[D + n0:D + n0 + NG]
        bqk = np.concatenate(
            [bq.reshape(4, 128).T, bk.reshape(4, 128).T], axis=1)  # [128, 8]
        in_maps.append({
            "x": np.ascontiguousarray(x[b]),
            "wq": np.ascontiguousarray(W_qkv[:, n0:n0 + NG]),
            "wk": np.ascontiguousarray(W_qkv[:, D + n0:D + n0 + NG]),
            "wv": np.ascontiguousarray(W_qkv[:, 2 * D + n0:2 * D + n0 + NG]),
            "bqk": np.ascontiguousarray(bqk),
            "bv": np.ascontiguousarray(
                b_qkv[2 * D + n0:2 * D + n0 + NG].reshape(1, NG)),
            "wo": np.ascontiguousarray(W_o[n0:n0 + NG, :]),
            "onesr": np.ones((1, 128), dtype=np.float32),
            "mask": _causal_masks(),
        })

    _CACHE["in_maps"] = in_maps
    res = run_bass_kernel_spmd(nc, in_maps, list(range(N_CORES)))
    outs = res.results

    out = np.empty((B, S, D), dtype=np.float32)
    for b in range(B):
        out[b] = outs[G * b]["out"] + outs[G * b + 1]["out"]
    out += b_o[None, None, :]
    return out
